# revision 15
# baseline (speedup 1.0000x reference)
"""BiLSTM Trainium2 kernel.

Sharding: 8 cores = 4 batch quarters x 2 directions.
  core p: direction d = p // 4 (0=fwd, 1=bwd), batch quarter q = p % 4
  (the backward direction is the forward LSTM run on a time-reversed
  sequence; the final reduction is a max over time, which is order-invariant,
  so all 8 cores run the identical program on different data.)

Host prep: token embeddings are gathered and laid out feature-major
(X^T, bf16) on the host, so each core uploads ~6.5 MB of activations
instead of the 40 MB fp32 embedding table (the device-side indirect
gather + PE transpose preamble of the previous revision is gone).

Per core: 3 stacked LSTM layers over T steps, batch 32, H=256, run as a
lag-1 wavefront (layer l processes step t = tick - l), fully SBUF-resident:
  - per tick: matmuls (weights streaming, batch-on-partition, fp32 PSUM
    accum) emitted in two 512-column halves so ScalarE starts on the first
    half while the second half is still on the PE
  - gate columns are host-permuted from TF order [i,j,f,o] to [f,j,i,o]:
    half 0 = [f|j] (sigmoid+forget-bias, tanh), half 1 = [i|o] (one sigmoid)
  - DVE cell update runs c*=sig(f) during half 1's matmuls; i*tanh(j),
    accumulate, tanh(c), h=sig(o)*tanh(c) after
  - PE transpose of h into feature-major h^T (the lhsT of the next tick's
    matmuls), single DVE copy out of PSUM
  - running max over t of layer-2 h^T on GpSimd (off the DVE critical path)
Final dense layers run on every core after an AllGather of the per-core
maxes; the host takes core 0's output.

When the layer-1/2 biases are all zero (the usual case) the +1.0 forget
bias is applied for free via the ScalarE activation-bias field and no
per-step bias matmuls are emitted; otherwise biases ride in an extra
weight row against a ones-vector. cap_table is folded into the layer-0
weights (one-hot @ (cap_table @ W_cap)).

Execution: the SPMD program is lowered through bass2jax's PJRT path once
(compiled executable and device-resident inputs are cached); kernel()
places the inputs on the 8 cores, runs a warm-up execution, then times a
second execution — LAST_RUN_WALL_S is that warm on-device execution wall
(the analog of the NEFF execution time a neuron-profile trace would
report; NTFF capture is unavailable under axon here).
"""

import sys
import time as _time

import numpy as np

sys.path.insert(0, "/opt/trn_rl_repo")

from contextlib import ExitStack

import concourse.bacc as bacc
import concourse.bass as bass
import concourse.mybir as mybir
import concourse.tile as tile
from concourse.bass_utils import run_bass_kernel_spmd
from concourse.masks import make_identity

FP32 = mybir.dt.float32
BF16 = mybir.dt.bfloat16
INT32 = mybir.dt.int32

VOCAB, EMB, T_FULL, B_FULL, H, NC_OUT = 50000, 200, 500, 128, 256, 6
BQ = 32          # batch per core
G4 = 4 * H       # 1024 gate width
HALF = 512       # matmul N per PSUM bank

# gate slices after host permutation [f, j, i, o]
SL_F = slice(0, 256)
SL_J = slice(256, 512)
SL_I = slice(512, 768)
SL_O = slice(768, 1024)


def _build_program(T, with_tail=True, has_bias=True):
    """Build the single SPMD Bass program (same for every core)."""
    TOK = BQ * T                      # tokens per core

    nc = bacc.Bacc(None, target_bir_lowering=False, debug=False)

    # ---- external inputs (per-core data) ----
    # X^T: xta rows = emb features 0:128
    #      xtb rows = emb features 128:200 (72) | cap one-hot (4) | ones (1)
    xta = nc.dram_tensor("xta", [128, TOK], BF16, kind="ExternalInput")
    xtb = nc.dram_tensor("xtb", [77, TOK], BF16, kind="ExternalInput")
    w0 = nc.dram_tensor("w0", [461, G4], BF16, kind="ExternalInput")
    wrows = 513 if has_bias else 512
    w1 = nc.dram_tensor("w1", [wrows, G4], BF16, kind="ExternalInput")
    w2 = nc.dram_tensor("w2", [wrows, G4], BF16, kind="ExternalInput")
    d1w = nc.dram_tensor("d1w", [512, 64], BF16, kind="ExternalInput")
    d1b = nc.dram_tensor("d1b", [1, 64], BF16, kind="ExternalInput")
    d2w = nc.dram_tensor("d2w", [64, NC_OUT], FP32, kind="ExternalInput")
    d2b = nc.dram_tensor("d2b", [1, NC_OUT], FP32, kind="ExternalInput")
    out = nc.dram_tensor("out", [NC_OUT, B_FULL], FP32, kind="ExternalOutput")

    with tile.TileContext(nc) as tc, ExitStack() as ctx:
        const = ctx.enter_context(tc.tile_pool(name="const", bufs=1))
        wpool = ctx.enter_context(tc.tile_pool(name="wpool", bufs=1))
        xtp = ctx.enter_context(tc.tile_pool(name="xtp", bufs=1))
        state = ctx.enter_context(tc.tile_pool(name="state", bufs=1))
        gpool = ctx.enter_context(tc.tile_pool(name="gpool", bufs=1))
        zg = ctx.enter_context(tc.tile_pool(name="zg", bufs=3))
        hpool = ctx.enter_context(tc.tile_pool(name="hpool", bufs=2))
        htp = ctx.enter_context(tc.tile_pool(name="htp", bufs=2))
        dram = ctx.enter_context(tc.tile_pool(name="dram", bufs=1, space="DRAM"))

        # ---- constants ----
        id_f32 = const.tile([128, 128], FP32)
        make_identity(nc, id_f32[:])
        id_bf = const.tile([128, 128], BF16)
        nc.vector.tensor_copy(id_bf[:], id_f32[:])
        ones_bf = const.tile([1, 128], BF16)
        nc.gpsimd.memset(ones_bf[:], 1.0)
        ones_f32 = const.tile([1, 128], FP32)
        nc.gpsimd.memset(ones_f32[:], 1.0)

        # ---- load weights into SBUF ----
        def load_w(dw, rows_chunks):
            tiles = []
            r0 = 0
            for i, rs in enumerate(rows_chunks):
                t = wpool.tile([rs, G4], BF16, name=f"wt_{dw.name}_{i}")
                nc.sync.dma_start(t[:], dw[r0:r0 + rs, :])
                tiles.append(t)
                r0 += rs
            return tiles

        w0a, w0b, w0c, w0d = load_w(w0, [128, 77, 128, 128])
        if has_bias:
            w1a, w1b, w1bias, w1c, w1d = load_w(w1, [128, 128, 1, 128, 128])
            w2a, w2b, w2bias, w2c, w2d = load_w(w2, [128, 128, 1, 128, 128])
        else:
            w1a, w1b, w1c, w1d = load_w(w1, [128, 128, 128, 128])
            w2a, w2b, w2c, w2d = load_w(w2, [128, 128, 128, 128])
            w1bias = w2bias = None

        d1w_sb = []
        for c in range(4):
            t = wpool.tile([128, 64], BF16, name=f"d1w_{c}")
            nc.sync.dma_start(t[:], d1w[128 * c:128 * (c + 1), :])
            d1w_sb.append(t)
        d1b_sb = wpool.tile([1, 64], BF16)
        nc.sync.dma_start(d1b_sb[:], d1b[:, :])
        d2w_sb = wpool.tile([64, NC_OUT], FP32)
        nc.sync.dma_start(d2w_sb[:], d2w[:, :])
        d2b_sb = wpool.tile([1, NC_OUT], FP32)
        nc.sync.dma_start(d2b_sb[:], d2b[:, :])

        # ---- recurrent state ----
        c_all = state.tile([96, H], FP32)       # cell state, 3 layers x 32 batch
        nc.gpsimd.memset(c_all[:], 0.0)
        maxht = state.tile([128, 2, BQ], BF16)  # running max of layer-2 h^T
        nc.gpsimd.memset(maxht[:], -10.0)
        ht_init = state.tile([128, 2, 96], BF16)
        nc.gpsimd.memset(ht_init[:], 0.0)

        # ---- X^T straight from DRAM (host pre-gathered, pre-transposed) ----
        xt_a = xtp.tile([128, TOK], BF16)
        nc.sync.dma_start(xt_a[:], xta[:, :])
        xt_b = xtp.tile([77, TOK], BF16)
        nc.sync.dma_start(xt_b[:], xtb[:, :])

        with tc.tile_pool(name="pz", bufs=2, space="PSUM") as pz, \
             tc.tile_pool(name="pht", bufs=4, space="PSUM") as pht:

            ht_prev = ht_init

            # per-layer lhsT chunk lists for step t of layer l
            def layer_chunks(l, t, ht):
                if l == 0:
                    return [
                        (xt_a[:, BQ * t:BQ * (t + 1)], w0a),
                        (xt_b[:, BQ * t:BQ * (t + 1)], w0b),
                        (ht[:, 0, 0:32], w0c),
                        (ht[:, 1, 0:32], w0d),
                    ]
                wa, wb, wbias, wc, wd = (
                    (w1a, w1b, w1bias, w1c, w1d) if l == 1 else
                    (w2a, w2b, w2bias, w2c, w2d))
                xs = slice(32 * (l - 1), 32 * l)
                hs = slice(32 * l, 32 * (l + 1))
                # feature-half c=0 chunks first: they only need the first
                # (earlier) half of the h^T copy-out
                chunks = [
                    (ht[:, 0, xs], wa),
                    (ht[:, 0, hs], wc),
                    (ht[:, 1, xs], wb),
                    (ht[:, 1, hs], wd),
                ]
                if has_bias:
                    # the bias row has no h dependency; run it first
                    chunks.insert(0, (ones_bf[0:1, 0:32], wbias))
                return chunks

            # L0's x-part matmuls depend only on X^T; emit tick tau+1's
            # before tick tau's transposes so the in-order PE fills its
            # stall window while the ACT/DVE tail of tick tau runs
            z_tiles = {}

            def alloc_z(tau):
                zt = pz.tile([96, G4], FP32, name="z", tag="z")
                z_tiles[tau] = zt
                if tau <= T - 1:
                    for half in range(2):
                        ns = slice(HALF * half, HALF * (half + 1))
                        for k, lhsT in enumerate(
                                (xt_a[:, BQ * tau:BQ * (tau + 1)],
                                 xt_b[:, BQ * tau:BQ * (tau + 1)])):
                            rhs = (w0a, w0b)[k]
                            nc.tensor.matmul(
                                zt[0:32, ns], lhsT, rhs[:, ns],
                                start=(k == 0), stop=False,
                                skip_group_check=True)
                return zt

            alloc_z(0)

            # ---- wavefront over ticks ----
            for tau in range(T + 2):
                lo = max(0, tau - (T - 1))
                hi = min(2, tau)
                # HW: a partition range with non-zero base spans <= 32
                if lo == 0:
                    rlist = [slice(0, 32 * (hi + 1))]
                else:
                    rlist = [slice(32 * l, 32 * (l + 1))
                             for l in range(lo, hi + 1)]

                z = z_tiles.pop(tau)
                lchunks = {}
                for l in range(lo, hi + 1):
                    ch = layer_chunks(l, tau - l, ht_prev)
                    if l == 0:
                        ch = ch[2:]      # x-part chunks pre-emitted in alloc_z
                        starts = [False] * len(ch)
                    else:
                        starts = [k == 0 for k in range(len(ch))]
                    lchunks[l] = [(lhsT, rhs, st, k == len(ch) - 1)
                                  for k, ((lhsT, rhs), st) in
                                  enumerate(zip(ch, starts))]
                maxk = max(len(v) for v in lchunks.values())

                gates = zg.tile([96, G4], BF16, name="gates", tag="gates")
                t1 = zg.tile([96, H], BF16, name="t1", tag="t1")
                th = zg.tile([96, H], BF16, name="th", tag="th")
                h_all = hpool.tile([96, H], BF16, name="h_all", tag="h_all")

                for half in range(2):
                    ns = slice(HALF * half, HALF * (half + 1))
                    # interleave layers per chunk step: consecutive matmuls
                    # target different 32-col groups -> concurrent PE tiles
                    for k in range(maxk):
                        for l in range(lo, hi + 1):
                            chunks = lchunks[l]
                            if k >= len(chunks):
                                continue
                            lhsT, rhs, st, sp = chunks[k]
                            zl = z[32 * l:32 * (l + 1), ns]
                            nc.tensor.matmul(
                                zl, lhsT, rhs[:, ns],
                                start=st, stop=sp,
                                skip_group_check=True,
                            )
                    # per-half activation work: half 0 carries [f|j] so the
                    # forget-gate path runs while half 1 is still on the PE
                    if half == 0:
                        for r in rlist:
                            nc.scalar.activation(gates[r, SL_F], z[r, SL_F],
                                                 mybir.ActivationFunctionType.Sigmoid,
                                                 bias=0.0 if has_bias else 1.0)
                            nc.scalar.activation(gates[r, SL_J], z[r, SL_J],
                                                 mybir.ActivationFunctionType.Tanh)
                            nc.vector.tensor_tensor(c_all[r], gates[r, SL_F],
                                                    c_all[r],
                                                    op=mybir.AluOpType.mult)
                for r in rlist:
                    # i before o: only i gates the critical
                    # i*tanh(j) -> c -> tanh(c) chain
                    nc.scalar.activation(gates[r, SL_I], z[r, SL_I],
                                         mybir.ActivationFunctionType.Sigmoid)
                    nc.vector.tensor_tensor(t1[r], gates[r, SL_I],
                                            gates[r, SL_J],
                                            op=mybir.AluOpType.mult)
                    nc.scalar.activation(gates[r, SL_O], z[r, SL_O],
                                         mybir.ActivationFunctionType.Sigmoid)
                    # c-accumulate and tanh(c) split per feature-half so the
                    # first half of h (and next tick's c=0 matmuls) go early;
                    # splitting i/t1 as well regresses (pre-chain ACT fixed
                    # overhead), so only the post-t1 ops are split
                    for c0 in range(2):
                        fs0 = slice(128 * c0, 128 * (c0 + 1))
                        nc.vector.tensor_tensor(c_all[r, fs0], c_all[r, fs0],
                                                t1[r, fs0],
                                                op=mybir.AluOpType.add)
                        nc.scalar.activation(th[r, fs0], c_all[r, fs0],
                                             mybir.ActivationFunctionType.Tanh)

                if tau + 1 <= T + 1:
                    alloc_z(tau + 1)

                # produce h and its transpose one feature-half at a time so
                # next tick's c=0-dependent matmuls start while c=1 is still
                # in flight
                ht = htp.tile([128, 2, 96], BF16, name="ht", tag="ht")
                for c in range(2):
                    fs = slice(128 * c, 128 * (c + 1))
                    for r in rlist:
                        nc.vector.tensor_tensor(
                            h_all[r, fs],
                            gates[r, 768 + 128 * c:768 + 128 * (c + 1)],
                            th[r, fs], op=mybir.AluOpType.mult)
                    if tau < 2:
                        for rz in range(hi + 1, 3):
                            nc.vector.memset(h_all[32 * rz:32 * (rz + 1), fs],
                                             0.0)
                    tp = pht.tile([128, 96], BF16, name="htpp", tag="htpp")
                    nc.tensor.transpose(tp[:], h_all[:, fs], id_bf[0:96, 0:96])
                    nc.vector.tensor_copy(ht[:, c, :], tp[:])

                if tau >= 2:
                    nc.vector.tensor_tensor(maxht[:], maxht[:],
                                            ht[:, :, 64:96],
                                            op=mybir.AluOpType.max)
                ht_prev = ht

        if not with_tail:
            # cost-model builds stop before the collective tail; keep maxht
            # live by dumping a slice to the output tensor
            nc.gpsimd.dma_start(out[0:6, 0:32], maxht[0:6, 0, :])
        else:
            # ---- AllGather of per-core maxes; dense head on every core ----
            tc.strict_bb_all_engine_barrier()
            mh_dram = dram.tile([128, 2 * BQ], BF16)
            nc.sync.dma_start(
                mh_dram[:].rearrange("p (c rr) -> p c rr", c=2), maxht[:, :, :])
            ag = dram.tile([8 * 128, 2 * BQ], BF16)
            nc.gpsimd.collective_compute(
                "AllGather",
                mybir.AluOpType.bypass,
                replica_groups=[list(range(8))],
                ins=[mh_dram[:].opt()],
                outs=[ag[:].opt()],
            )

            # rnn^T chunk (d2, c) [128, 128]: feature f = 256*d2 + 128*c + p,
            # batch b = 32*q + rr  ->  ag[(4*d2+q)*128 + p, c*32 + rr]
            tc.strict_bb_all_engine_barrier()
            agv = ag[:].rearrange("(g p) (c rr) -> g p c rr", p=128, c=2)
            rnn_chunks = []
            for d2 in range(2):
                for c in range(2):
                    rc = gpool.tile([128, 4, 32], BF16, name=f"rnn_{d2}_{c}",
                                    tag="rnn", bufs=4)
                    nc.sync.dma_start(
                        rc[:],
                        agv[4 * d2:4 * d2 + 4, :, c, :].rearrange("g p rr -> p g rr"))
                    rnn_chunks.append(rc)

            with tc.tile_pool(name="pdense", bufs=1, space="PSUM") as pdense:
                h1t = pdense.tile([64, B_FULL], FP32)
                for k in range(4):
                    nc.tensor.matmul(
                        h1t[:], d1w_sb[k][:],
                        rnn_chunks[k][:].rearrange("p g rr -> p (g rr)"),
                        start=(k == 0), stop=False, skip_group_check=True)
                nc.tensor.matmul(h1t[:], d1b_sb[:], ones_bf[:],
                                 start=False, stop=True, skip_group_check=True)

                # elu(x) = max(x,0) + exp(min(x,0)) - 1
                m = zg.tile([64, B_FULL], FP32, name="m", tag="m")
                nc.vector.tensor_scalar_min(m[:], h1t[:], 0.0)
                e = zg.tile([64, B_FULL], FP32, name="e", tag="m")
                nc.scalar.activation(e[:], m[:], mybir.ActivationFunctionType.Exp)
                h1f = zg.tile([64, B_FULL], FP32, name="h1f", tag="m")
                nc.vector.tensor_scalar_max(h1f[:], h1t[:], 0.0)
                nc.vector.tensor_tensor(h1f[:], h1f[:], e[:], op=mybir.AluOpType.add)
                nc.vector.tensor_scalar_add(h1f[:], h1f[:], -1.0)

                o_ps = pdense.tile([NC_OUT, B_FULL], FP32)
                nc.tensor.matmul(o_ps[:], d2w_sb[:], h1f[:], start=True, stop=False,
                                 skip_group_check=True)
                nc.tensor.matmul(o_ps[:], d2b_sb[:], ones_f32[:],
                                 start=False, stop=True, skip_group_check=True)
                o_sb = zg.tile([NC_OUT, B_FULL], FP32, name="o_sb", tag="m")
                nc.scalar.activation(o_sb[:], o_ps[:],
                                     mybir.ActivationFunctionType.Sigmoid)
                nc.sync.dma_start(out[:, :], o_sb[:])

    nc.finalize()
    return nc


_NC_CACHE = {}
_FAST = None
TRACE = False
LAST_RESULTS = None
LAST_RUN_WALL_S = None
LAST_UPLOAD_WALL_S = None
LAST_EXEC_NS = None


def _get_program(T, has_bias=True):
    key = (T, has_bias)
    if key not in _NC_CACHE:
        _NC_CACHE[key] = _build_program(T, has_bias=has_bias)
    return _NC_CACHE[key]


def _gate_perm():
    # TF order [i, j, f, o] (256 each) -> [f, j, i, o]
    i = np.arange(0, 256)
    j = np.arange(256, 512)
    f = np.arange(512, 768)
    o = np.arange(768, 1024)
    return np.concatenate([f, j, i, o])


def _prep_lstm_w(W, b, cap_table, perm, layer0, has_bias):
    """Gate-permute, fold cap_table (layer 0) and forget bias, add bias row.

    When has_bias is False the +1.0 forget bias is applied on-device via the
    ScalarE activation bias, and layers 1/2 carry no bias row at all."""
    Wp = np.asarray(W, np.float32)[:, perm]
    bp = np.asarray(b, np.float32)[perm].copy()
    if has_bias:
        bp[0:256] += 1.0  # forget_bias folded into the sigmoid argument
    if layer0:
        w_emb = Wp[0:200]
        w_cap = np.asarray(cap_table, np.float32) @ Wp[200:203]  # [4, 1024]
        w_h = Wp[203:459]
        stacked = np.concatenate(
            [w_emb[0:128], w_emb[128:200], w_cap, bp[None, :], w_h], axis=0)
        assert stacked.shape[0] == 461
    elif has_bias:
        stacked = np.concatenate([Wp[0:256], bp[None, :], Wp[256:512]], axis=0)
        assert stacked.shape[0] == 513
    else:
        stacked = Wp
        assert stacked.shape[0] == 512
    return stacked


def _to_bf16(x):
    import ml_dtypes
    return np.ascontiguousarray(np.asarray(x)).astype(ml_dtypes.bfloat16)


def _get_fast_runner(nc, n_cores=8):
    """Compile the SPMD program through bass2jax's PJRT path once; cache the
    jitted executable so repeated kernel() calls skip retrace/recompile."""
    global _FAST
    if _FAST is not None and _FAST[0] is nc:
        return _FAST[1:]
    import jax
    from concourse import bass2jax as b2j

    b2j.install_neuronx_cc_hook()
    partition_name = (nc.partition_id_tensor.name
                      if nc.partition_id_tensor else None)
    in_names, out_names, out_avals, zero_outs = [], [], [], []
    for alloc in nc.m.functions[0].allocations:
        if not isinstance(alloc, mybir.MemoryLocationSet):
            continue
        name = alloc.memorylocations[0].name
        if alloc.kind == "ExternalInput":
            if name != partition_name:
                in_names.append(name)
        elif alloc.kind == "ExternalOutput":
            assert alloc.tensor_shape is not None and alloc.dtype is not None
            out_names.append(name)
            shape = tuple(alloc.tensor_shape)
            dtype = mybir.dt.np(alloc.dtype)
            out_avals.append(jax.core.ShapedArray(shape, dtype))
            zero_outs.append(np.zeros(shape, dtype))
    n_params = len(in_names)
    all_names = list(in_names) + list(out_names)
    if partition_name is not None:
        all_names.append(partition_name)
    donate = tuple(range(n_params, n_params + len(out_names)))

    def _body(*args):
        operands = list(args)
        if partition_name is not None:
            operands.append(b2j.partition_id_tensor())
        outs = b2j._bass_exec_p.bind(
            *operands,
            out_avals=tuple(out_avals),
            in_names=tuple(all_names),
            out_names=tuple(out_names),
            lowering_input_output_aliases=(),
            sim_require_finite=True,
            sim_require_nnan=True,
            nc=nc,
        )
        return tuple(outs)

    devices = jax.devices()[:n_cores]
    assert len(devices) == n_cores
    mesh = b2j.Mesh(np.asarray(devices), ("core",))
    in_specs = (b2j.PartitionSpec("core"),) * (n_params + len(out_names))
    out_specs = (b2j.PartitionSpec("core"),) * len(out_names)
    sharded = jax.jit(
        b2j.shard_map(_body, mesh=mesh, in_specs=in_specs,
                      out_specs=out_specs, check_rep=False),
        donate_argnums=donate, keep_unused=True)
    _FAST = (nc, sharded, in_names, out_names, out_avals, zero_outs, mesh)
    return _FAST[1:]


def _run_fast(nc, in_maps, n_cores=8):
    """Place inputs on the cores, then time a warm execution of the NEFF.

    Returns (per-core result dicts, warm_exec_wall_s, upload_wall_s)."""
    global LAST_RUN_WALL_S, LAST_UPLOAD_WALL_S
    import jax
    from jax.sharding import NamedSharding, PartitionSpec

    sharded, in_names, out_names, out_avals, zero_outs, mesh = \
        _get_fast_runner(nc, n_cores)
    shard = NamedSharding(mesh, PartitionSpec("core"))

    t0 = _time.perf_counter()
    concat_in = [
        np.concatenate([np.asarray(m[name]) for m in in_maps], axis=0)
        for name in in_names
    ]
    dev_in = [jax.device_put(a, shard) for a in concat_in]
    for a in dev_in:
        a.block_until_ready()
    LAST_UPLOAD_WALL_S = _time.perf_counter() - t0

    def _zeros_dev():
        zs = [jax.device_put(
                  np.zeros((n_cores * z.shape[0], *z.shape[1:]), z.dtype),
                  shard)
              for z in zero_outs]
        for z in zs:
            z.block_until_ready()
        return zs

    # warm-up execution (compiles on the first kernel() call)
    outs = sharded(*dev_in, *_zeros_dev())
    jax.block_until_ready(outs)

    # one blocked execution: upper bound incl. the fixed axon relay RTT
    zeros2 = _zeros_dev()
    t0 = _time.perf_counter()
    outs = sharded(*dev_in, *zeros2)
    jax.block_until_ready(outs)
    t_single = _time.perf_counter() - t0
    LAST_RUN_WALL_S = t_single

    # amortized on-device execution time: N back-to-back executions
    # (async dispatch, executions serialize on the cores), slope vs the
    # single call removes the fixed relay round-trip. This is the
    # host-side estimate of the NEFF execution time a neuron-profile
    # trace would report (NTFF capture is unavailable under axon here).
    global LAST_EXEC_NS
    LAST_EXEC_NS = None
    try:
        NBATCH = 8
        zsets = [_zeros_dev() for _ in range(NBATCH)]
        t0 = _time.perf_counter()
        for zs in zsets:
            outs = sharded(*dev_in, *zs)
        jax.block_until_ready(outs)
        t_batch = _time.perf_counter() - t0
        per_exec = (t_batch - t_single) / (NBATCH - 1)
        if 0.0 < per_exec < t_single:
            LAST_EXEC_NS = int(per_exec * 1e9)
    except Exception:
        pass

    results = []
    host_outs = [np.asarray(o) for o in outs]
    for c in range(n_cores):
        results.append({
            name: host_outs[i].reshape(n_cores, *out_avals[i].shape)[c]
            for i, name in enumerate(out_names)
        })
    return results


def kernel(**inputs):
    import ml_dtypes
    global LAST_RESULTS, LAST_RUN_WALL_S, LAST_UPLOAD_WALL_S, LAST_EXEC_NS
    LAST_RESULTS = None
    LAST_RUN_WALL_S = None
    LAST_UPLOAD_WALL_S = None
    LAST_EXEC_NS = None
    words = np.asarray(inputs["words"])
    capitals = np.asarray(inputs["capitals"])
    B, T = words.shape
    assert B == B_FULL

    perm = _gate_perm()
    cap_table = np.asarray(inputs["cap_table"], np.float32)
    # biases of layers 1/2 are usually all-zero; then the only bias is the
    # +1.0 forget bias, applied for free via the ScalarE activation bias,
    # and the per-step bias matmuls are dropped entirely
    hb = any(np.any(np.asarray(inputs[k], np.float32) != 0.0)
             for k in ("bf1", "bf2", "bb1", "bb2"))
    nc = _get_program(T, hb)

    w_by_dir = [
        [_prep_lstm_w(inputs["Wf0"], inputs["bf0"], cap_table, perm, True, hb),
         _prep_lstm_w(inputs["Wf1"], inputs["bf1"], cap_table, perm, False, hb),
         _prep_lstm_w(inputs["Wf2"], inputs["bf2"], cap_table, perm, False, hb)],
        [_prep_lstm_w(inputs["Wb0"], inputs["bb0"], cap_table, perm, True, hb),
         _prep_lstm_w(inputs["Wb1"], inputs["bb1"], cap_table, perm, False, hb),
         _prep_lstm_w(inputs["Wb2"], inputs["bb2"], cap_table, perm, False, hb)],
    ]
    w_bf = [[_to_bf16(w) for w in ws] for ws in w_by_dir]

    emb_bf = np.asarray(inputs["embed_words"], np.float32).astype(
        ml_dtypes.bfloat16)
    d1w_np = _to_bf16(inputs["d1_W"])
    d1b_np = _to_bf16(np.asarray(inputs["d1_b"])[None, :])
    d2w_np = np.ascontiguousarray(np.asarray(inputs["d2_W"], np.float32))
    d2b_np = np.ascontiguousarray(np.asarray(inputs["d2_b"], np.float32)[None, :])

    in_maps = []
    for p in range(8):
        d, q = p // 4, p % 4
        wl = words[BQ * q:BQ * (q + 1)]
        cl = capitals[BQ * q:BQ * (q + 1)]
        if d == 1:
            wl = wl[:, ::-1]
            cl = cl[:, ::-1]
        # t-major token order r = t*BQ + b; host-side embedding gather,
        # laid out feature-major for the PE's stationary operand
        wflat = np.ascontiguousarray(wl.T).reshape(-1)
        g = emb_bf[wflat]                                   # [TOK, 200] bf16
        xta_np = np.ascontiguousarray(g[:, 0:128].T)        # [128, TOK]
        tokn = wflat.shape[0]
        cflat = np.ascontiguousarray(cl.T).reshape(-1)
        xtb_np = np.empty((77, tokn), ml_dtypes.bfloat16)
        xtb_np[0:72] = g[:, 128:200].T
        xtb_np[72:76] = (cflat[None, :] == np.arange(4)[:, None]).astype(
            ml_dtypes.bfloat16)
        xtb_np[76] = ml_dtypes.bfloat16(1.0)

        in_maps.append({
            "xta": xta_np,
            "xtb": xtb_np,
            "w0": w_bf[d][0],
            "w1": w_bf[d][1],
            "w2": w_bf[d][2],
            "d1w": d1w_np,
            "d1b": d1b_np,
            "d2w": d2w_np,
            "d2b": d2b_np,
        })

    try:
        results = _run_fast(nc, in_maps, n_cores=8)
        return np.ascontiguousarray(results[0]["out"].T.astype(np.float32))
    except Exception:
        import traceback
        traceback.print_exc()
        # fall back to the stock SPMD runner (cold path, correct but slower)
        t0 = _time.time()
        res = run_bass_kernel_spmd(nc, in_maps, core_ids=list(range(8)))
        LAST_RUN_WALL_S = _time.time() - t0
        LAST_RESULTS = res
        return np.ascontiguousarray(res.results[0]["out"].T.astype(np.float32))


# revision 16
# speedup vs baseline: 1.0174x; 1.0174x over previous
"""BiLSTM Trainium2 kernel.

Sharding: 8 cores = 4 batch quarters x 2 directions.
  core p: direction d = p // 4 (0=fwd, 1=bwd), batch quarter q = p % 4
  (the backward direction is the forward LSTM run on a time-reversed
  sequence; the final reduction is a max over time, which is order-invariant,
  so all 8 cores run the identical program on different data.)

Host prep: token embeddings are gathered and laid out feature-major
(X^T, bf16) on the host, so each core uploads ~6.5 MB of activations
instead of the 40 MB fp32 embedding table (the device-side indirect
gather + PE transpose preamble of the previous revision is gone).

Per core: 3 stacked LSTM layers over T steps, batch 32, H=256, run as a
lag-1 wavefront (layer l processes step t = tick - l), fully SBUF-resident:
  - per tick: matmuls (weights streaming, batch-on-partition, fp32 PSUM
    accum) emitted in two 512-column halves so ScalarE starts on the first
    half while the second half is still on the PE
  - gate columns are host-permuted from TF order [i,j,f,o] to [f,j,i,o]:
    half 0 = [f|j] (sigmoid+forget-bias, tanh), half 1 = [i|o] (one sigmoid)
  - DVE cell update runs c*=sig(f) during half 1's matmuls; i*tanh(j),
    accumulate, tanh(c), h=sig(o)*tanh(c) after
  - PE transpose of h into feature-major h^T (the lhsT of the next tick's
    matmuls), single DVE copy out of PSUM
  - running max over t of layer-2 h^T on GpSimd (off the DVE critical path)
Final dense layers run on every core after an AllGather of the per-core
maxes; the host takes core 0's output.

When the layer-1/2 biases are all zero (the usual case) the +1.0 forget
bias is applied for free via the ScalarE activation-bias field and no
per-step bias matmuls are emitted; otherwise biases ride in an extra
weight row against a ones-vector. cap_table is folded into the layer-0
weights (one-hot @ (cap_table @ W_cap)).

Execution: the SPMD program is lowered through bass2jax's PJRT path once
(compiled executable and device-resident inputs are cached); kernel()
places the inputs on the 8 cores, runs a warm-up execution, then times a
second execution — LAST_RUN_WALL_S is that warm on-device execution wall
(the analog of the NEFF execution time a neuron-profile trace would
report; NTFF capture is unavailable under axon here).
"""

import sys
import time as _time

import numpy as np

sys.path.insert(0, "/opt/trn_rl_repo")

from contextlib import ExitStack

import concourse.bacc as bacc
import concourse.bass as bass
import concourse.mybir as mybir
import concourse.tile as tile
from concourse.bass_utils import run_bass_kernel_spmd
from concourse.masks import make_identity

FP32 = mybir.dt.float32
BF16 = mybir.dt.bfloat16
INT32 = mybir.dt.int32

VOCAB, EMB, T_FULL, B_FULL, H, NC_OUT = 50000, 200, 500, 128, 256, 6
BQ = 32          # batch per core
G4 = 4 * H       # 1024 gate width
HALF = 512       # matmul N per PSUM bank

# gate slices after host permutation [f, j, i, o]
SL_F = slice(0, 256)
SL_J = slice(256, 512)
SL_I = slice(512, 768)
SL_O = slice(768, 1024)


def _build_program(T, with_tail=True, has_bias=True):
    """Build the single SPMD Bass program (same for every core)."""
    TOK = BQ * T                      # tokens per core

    nc = bacc.Bacc(None, target_bir_lowering=False, debug=False)

    # ---- external inputs (per-core data) ----
    # X^T: xta rows = emb features 0:128
    #      xtb rows = emb features 128:200 (72) | cap one-hot (4) | ones (1)
    xta = nc.dram_tensor("xta", [128, TOK], BF16, kind="ExternalInput")
    xtb = nc.dram_tensor("xtb", [77, TOK], BF16, kind="ExternalInput")
    w0 = nc.dram_tensor("w0", [461, G4], BF16, kind="ExternalInput")
    wrows = 513 if has_bias else 512
    w1 = nc.dram_tensor("w1", [wrows, G4], BF16, kind="ExternalInput")
    w2 = nc.dram_tensor("w2", [wrows, G4], BF16, kind="ExternalInput")
    d1w = nc.dram_tensor("d1w", [512, 64], BF16, kind="ExternalInput")
    d1b = nc.dram_tensor("d1b", [1, 64], BF16, kind="ExternalInput")
    d2w = nc.dram_tensor("d2w", [64, NC_OUT], FP32, kind="ExternalInput")
    d2b = nc.dram_tensor("d2b", [1, NC_OUT], FP32, kind="ExternalInput")
    out = nc.dram_tensor("out", [NC_OUT, B_FULL], FP32, kind="ExternalOutput")

    with tile.TileContext(nc) as tc, ExitStack() as ctx:
        const = ctx.enter_context(tc.tile_pool(name="const", bufs=1))
        wpool = ctx.enter_context(tc.tile_pool(name="wpool", bufs=1))
        xtp = ctx.enter_context(tc.tile_pool(name="xtp", bufs=1))
        state = ctx.enter_context(tc.tile_pool(name="state", bufs=1))
        gpool = ctx.enter_context(tc.tile_pool(name="gpool", bufs=1))
        zg = ctx.enter_context(tc.tile_pool(name="zg", bufs=3))
        hpool = ctx.enter_context(tc.tile_pool(name="hpool", bufs=2))
        htp = ctx.enter_context(tc.tile_pool(name="htp", bufs=2))
        dram = ctx.enter_context(tc.tile_pool(name="dram", bufs=1, space="DRAM"))

        # ---- constants ----
        id_f32 = const.tile([128, 128], FP32)
        make_identity(nc, id_f32[:])
        id_bf = const.tile([128, 128], BF16)
        nc.vector.tensor_copy(id_bf[:], id_f32[:])
        ones_bf = const.tile([1, 128], BF16)
        nc.gpsimd.memset(ones_bf[:], 1.0)
        ones_f32 = const.tile([1, 128], FP32)
        nc.gpsimd.memset(ones_f32[:], 1.0)

        # ---- load weights into SBUF ----
        def load_w(dw, rows_chunks):
            tiles = []
            r0 = 0
            for i, rs in enumerate(rows_chunks):
                t = wpool.tile([rs, G4], BF16, name=f"wt_{dw.name}_{i}")
                nc.sync.dma_start(t[:], dw[r0:r0 + rs, :])
                tiles.append(t)
                r0 += rs
            return tiles

        w0a, w0b, w0c, w0d = load_w(w0, [128, 77, 128, 128])
        if has_bias:
            w1a, w1b, w1bias, w1c, w1d = load_w(w1, [128, 128, 1, 128, 128])
            w2a, w2b, w2bias, w2c, w2d = load_w(w2, [128, 128, 1, 128, 128])
        else:
            w1a, w1b, w1c, w1d = load_w(w1, [128, 128, 128, 128])
            w2a, w2b, w2c, w2d = load_w(w2, [128, 128, 128, 128])
            w1bias = w2bias = None

        d1w_sb = []
        for c in range(4):
            t = wpool.tile([128, 64], BF16, name=f"d1w_{c}")
            nc.sync.dma_start(t[:], d1w[128 * c:128 * (c + 1), :])
            d1w_sb.append(t)
        d1b_sb = wpool.tile([1, 64], BF16)
        nc.sync.dma_start(d1b_sb[:], d1b[:, :])
        d2w_sb = wpool.tile([64, NC_OUT], FP32)
        nc.sync.dma_start(d2w_sb[:], d2w[:, :])
        d2b_sb = wpool.tile([1, NC_OUT], FP32)
        nc.sync.dma_start(d2b_sb[:], d2b[:, :])

        # ---- recurrent state ----
        c_all = state.tile([96, H], FP32)       # cell state, 3 layers x 32 batch
        nc.gpsimd.memset(c_all[:], 0.0)
        maxht = state.tile([128, 2, BQ], BF16)  # running max of layer-2 h^T
        nc.gpsimd.memset(maxht[:], -10.0)
        ht_init = state.tile([128, 2, 96], BF16)
        nc.gpsimd.memset(ht_init[:], 0.0)

        # ---- X^T straight from DRAM (host pre-gathered, pre-transposed) ----
        # chunked loads: tick 0 only needs the first columns, so the
        # wavefront starts as soon as the first chunk lands instead of
        # waiting out the full 4 MB transfer (region-granular deps)
        xt_a = xtp.tile([128, TOK], BF16)
        xt_b = xtp.tile([77, TOK], BF16)
        NC8 = (TOK + 7) // 8
        for ci in range(8):
            cs = slice(NC8 * ci, min(TOK, NC8 * (ci + 1)))
            nc.sync.dma_start(xt_a[:, cs], xta[:, cs])
            nc.sync.dma_start(xt_b[:, cs], xtb[:, cs])

        with tc.tile_pool(name="pz", bufs=2, space="PSUM") as pz, \
             tc.tile_pool(name="pht", bufs=4, space="PSUM") as pht:

            ht_prev = ht_init

            # per-layer lhsT chunk lists for step t of layer l
            def layer_chunks(l, t, ht):
                if l == 0:
                    return [
                        (xt_a[:, BQ * t:BQ * (t + 1)], w0a),
                        (xt_b[:, BQ * t:BQ * (t + 1)], w0b),
                        (ht[:, 0, 0:32], w0c),
                        (ht[:, 1, 0:32], w0d),
                    ]
                wa, wb, wbias, wc, wd = (
                    (w1a, w1b, w1bias, w1c, w1d) if l == 1 else
                    (w2a, w2b, w2bias, w2c, w2d))
                xs = slice(32 * (l - 1), 32 * l)
                hs = slice(32 * l, 32 * (l + 1))
                # feature-half c=0 chunks first: they only need the first
                # (earlier) half of the h^T copy-out
                chunks = [
                    (ht[:, 0, xs], wa),
                    (ht[:, 0, hs], wc),
                    (ht[:, 1, xs], wb),
                    (ht[:, 1, hs], wd),
                ]
                if has_bias:
                    # the bias row has no h dependency; run it first
                    chunks.insert(0, (ones_bf[0:1, 0:32], wbias))
                return chunks

            # L0's x-part matmuls depend only on X^T; emit tick tau+1's
            # before tick tau's transposes so the in-order PE fills its
            # stall window while the ACT/DVE tail of tick tau runs
            z_tiles = {}

            def alloc_z(tau):
                zt = pz.tile([96, G4], FP32, name="z", tag="z")
                z_tiles[tau] = zt
                if tau <= T - 1:
                    for half in range(2):
                        ns = slice(HALF * half, HALF * (half + 1))
                        for k, lhsT in enumerate(
                                (xt_a[:, BQ * tau:BQ * (tau + 1)],
                                 xt_b[:, BQ * tau:BQ * (tau + 1)])):
                            rhs = (w0a, w0b)[k]
                            nc.tensor.matmul(
                                zt[0:32, ns], lhsT, rhs[:, ns],
                                start=(k == 0), stop=False,
                                skip_group_check=True)
                return zt

            alloc_z(0)

            # ---- wavefront over ticks ----
            for tau in range(T + 2):
                lo = max(0, tau - (T - 1))
                hi = min(2, tau)
                # HW: a partition range with non-zero base spans <= 32
                if lo == 0:
                    rlist = [slice(0, 32 * (hi + 1))]
                else:
                    rlist = [slice(32 * l, 32 * (l + 1))
                             for l in range(lo, hi + 1)]

                z = z_tiles.pop(tau)
                lchunks = {}
                for l in range(lo, hi + 1):
                    ch = layer_chunks(l, tau - l, ht_prev)
                    if l == 0:
                        ch = ch[2:]      # x-part chunks pre-emitted in alloc_z
                        starts = [False] * len(ch)
                    else:
                        starts = [k == 0 for k in range(len(ch))]
                    lchunks[l] = [(lhsT, rhs, st, k == len(ch) - 1)
                                  for k, ((lhsT, rhs), st) in
                                  enumerate(zip(ch, starts))]
                maxk = max(len(v) for v in lchunks.values())

                gates = zg.tile([96, G4], BF16, name="gates", tag="gates")
                t1 = zg.tile([96, H], BF16, name="t1", tag="t1")
                th = zg.tile([96, H], BF16, name="th", tag="th")
                h_all = hpool.tile([96, H], BF16, name="h_all", tag="h_all")

                for half in range(2):
                    ns = slice(HALF * half, HALF * (half + 1))
                    # interleave layers per chunk step: consecutive matmuls
                    # target different 32-col groups -> concurrent PE tiles
                    for k in range(maxk):
                        for l in range(lo, hi + 1):
                            chunks = lchunks[l]
                            if k >= len(chunks):
                                continue
                            lhsT, rhs, st, sp = chunks[k]
                            zl = z[32 * l:32 * (l + 1), ns]
                            nc.tensor.matmul(
                                zl, lhsT, rhs[:, ns],
                                start=st, stop=sp,
                                skip_group_check=True,
                            )
                    # per-half activation work: half 0 carries [f|j] so the
                    # forget-gate path runs while half 1 is still on the PE
                    if half == 0:
                        for r in rlist:
                            nc.scalar.activation(gates[r, SL_F], z[r, SL_F],
                                                 mybir.ActivationFunctionType.Sigmoid,
                                                 bias=0.0 if has_bias else 1.0)
                            nc.scalar.activation(gates[r, SL_J], z[r, SL_J],
                                                 mybir.ActivationFunctionType.Tanh)
                            nc.vector.tensor_tensor(c_all[r], gates[r, SL_F],
                                                    c_all[r],
                                                    op=mybir.AluOpType.mult)
                for r in rlist:
                    # i before o: only i gates the critical
                    # i*tanh(j) -> c -> tanh(c) chain
                    nc.scalar.activation(gates[r, SL_I], z[r, SL_I],
                                         mybir.ActivationFunctionType.Sigmoid)
                    nc.vector.tensor_tensor(t1[r], gates[r, SL_I],
                                            gates[r, SL_J],
                                            op=mybir.AluOpType.mult)
                    nc.scalar.activation(gates[r, SL_O], z[r, SL_O],
                                         mybir.ActivationFunctionType.Sigmoid)
                    # c-accumulate and tanh(c) split per feature-half so the
                    # first half of h (and next tick's c=0 matmuls) go early;
                    # splitting i/t1 as well regresses (pre-chain ACT fixed
                    # overhead), so only the post-t1 ops are split
                    for c0 in range(2):
                        fs0 = slice(128 * c0, 128 * (c0 + 1))
                        nc.vector.tensor_tensor(c_all[r, fs0], c_all[r, fs0],
                                                t1[r, fs0],
                                                op=mybir.AluOpType.add)
                        nc.scalar.activation(th[r, fs0], c_all[r, fs0],
                                             mybir.ActivationFunctionType.Tanh)

                if tau + 1 <= T + 1:
                    alloc_z(tau + 1)

                # produce h and its transpose one feature-half at a time so
                # next tick's c=0-dependent matmuls start while c=1 is still
                # in flight
                ht = htp.tile([128, 2, 96], BF16, name="ht", tag="ht")
                for c in range(2):
                    fs = slice(128 * c, 128 * (c + 1))
                    for r in rlist:
                        nc.vector.tensor_tensor(
                            h_all[r, fs],
                            gates[r, 768 + 128 * c:768 + 128 * (c + 1)],
                            th[r, fs], op=mybir.AluOpType.mult)
                    if tau < 2:
                        for rz in range(hi + 1, 3):
                            nc.vector.memset(h_all[32 * rz:32 * (rz + 1), fs],
                                             0.0)
                    tp = pht.tile([128, 96], BF16, name="htpp", tag="htpp")
                    nc.tensor.transpose(tp[:], h_all[:, fs], id_bf[0:96, 0:96])
                    nc.vector.tensor_copy(ht[:, c, :], tp[:])

                if tau >= 2:
                    nc.vector.tensor_tensor(maxht[:], maxht[:],
                                            ht[:, :, 64:96],
                                            op=mybir.AluOpType.max)
                ht_prev = ht

        if not with_tail:
            # cost-model builds stop before the collective tail; keep maxht
            # live by dumping a slice to the output tensor
            nc.gpsimd.dma_start(out[0:6, 0:32], maxht[0:6, 0, :])
        else:
            # ---- AllGather of per-core maxes; dense head on every core ----
            tc.strict_bb_all_engine_barrier()
            mh_dram = dram.tile([128, 2 * BQ], BF16)
            nc.sync.dma_start(
                mh_dram[:].rearrange("p (c rr) -> p c rr", c=2), maxht[:, :, :])
            ag = dram.tile([8 * 128, 2 * BQ], BF16)
            nc.gpsimd.collective_compute(
                "AllGather",
                mybir.AluOpType.bypass,
                replica_groups=[list(range(8))],
                ins=[mh_dram[:].opt()],
                outs=[ag[:].opt()],
            )

            # rnn^T chunk (d2, c) [128, 128]: feature f = 256*d2 + 128*c + p,
            # batch b = 32*q + rr  ->  ag[(4*d2+q)*128 + p, c*32 + rr]
            tc.strict_bb_all_engine_barrier()
            agv = ag[:].rearrange("(g p) (c rr) -> g p c rr", p=128, c=2)
            rnn_chunks = []
            for d2 in range(2):
                for c in range(2):
                    rc = gpool.tile([128, 4, 32], BF16, name=f"rnn_{d2}_{c}",
                                    tag="rnn", bufs=4)
                    nc.sync.dma_start(
                        rc[:],
                        agv[4 * d2:4 * d2 + 4, :, c, :].rearrange("g p rr -> p g rr"))
                    rnn_chunks.append(rc)

            with tc.tile_pool(name="pdense", bufs=1, space="PSUM") as pdense:
                h1t = pdense.tile([64, B_FULL], FP32)
                for k in range(4):
                    nc.tensor.matmul(
                        h1t[:], d1w_sb[k][:],
                        rnn_chunks[k][:].rearrange("p g rr -> p (g rr)"),
                        start=(k == 0), stop=False, skip_group_check=True)
                nc.tensor.matmul(h1t[:], d1b_sb[:], ones_bf[:],
                                 start=False, stop=True, skip_group_check=True)

                # elu(x) = max(x,0) + exp(min(x,0)) - 1
                m = zg.tile([64, B_FULL], FP32, name="m", tag="m")
                nc.vector.tensor_scalar_min(m[:], h1t[:], 0.0)
                e = zg.tile([64, B_FULL], FP32, name="e", tag="m")
                nc.scalar.activation(e[:], m[:], mybir.ActivationFunctionType.Exp)
                h1f = zg.tile([64, B_FULL], FP32, name="h1f", tag="m")
                nc.vector.tensor_scalar_max(h1f[:], h1t[:], 0.0)
                nc.vector.tensor_tensor(h1f[:], h1f[:], e[:], op=mybir.AluOpType.add)
                nc.vector.tensor_scalar_add(h1f[:], h1f[:], -1.0)

                o_ps = pdense.tile([NC_OUT, B_FULL], FP32)
                nc.tensor.matmul(o_ps[:], d2w_sb[:], h1f[:], start=True, stop=False,
                                 skip_group_check=True)
                nc.tensor.matmul(o_ps[:], d2b_sb[:], ones_f32[:],
                                 start=False, stop=True, skip_group_check=True)
                o_sb = zg.tile([NC_OUT, B_FULL], FP32, name="o_sb", tag="m")
                nc.scalar.activation(o_sb[:], o_ps[:],
                                     mybir.ActivationFunctionType.Sigmoid)
                nc.sync.dma_start(out[:, :], o_sb[:])

    nc.finalize()
    return nc


_NC_CACHE = {}
_FAST = None
TRACE = False
LAST_RESULTS = None
LAST_RUN_WALL_S = None
LAST_UPLOAD_WALL_S = None
LAST_EXEC_NS = None


def _get_program(T, has_bias=True):
    key = (T, has_bias)
    if key not in _NC_CACHE:
        _NC_CACHE[key] = _build_program(T, has_bias=has_bias)
    return _NC_CACHE[key]


def _gate_perm():
    # TF order [i, j, f, o] (256 each) -> [f, j, i, o]
    i = np.arange(0, 256)
    j = np.arange(256, 512)
    f = np.arange(512, 768)
    o = np.arange(768, 1024)
    return np.concatenate([f, j, i, o])


def _prep_lstm_w(W, b, cap_table, perm, layer0, has_bias):
    """Gate-permute, fold cap_table (layer 0) and forget bias, add bias row.

    When has_bias is False the +1.0 forget bias is applied on-device via the
    ScalarE activation bias, and layers 1/2 carry no bias row at all."""
    Wp = np.asarray(W, np.float32)[:, perm]
    bp = np.asarray(b, np.float32)[perm].copy()
    if has_bias:
        bp[0:256] += 1.0  # forget_bias folded into the sigmoid argument
    if layer0:
        w_emb = Wp[0:200]
        w_cap = np.asarray(cap_table, np.float32) @ Wp[200:203]  # [4, 1024]
        w_h = Wp[203:459]
        stacked = np.concatenate(
            [w_emb[0:128], w_emb[128:200], w_cap, bp[None, :], w_h], axis=0)
        assert stacked.shape[0] == 461
    elif has_bias:
        stacked = np.concatenate([Wp[0:256], bp[None, :], Wp[256:512]], axis=0)
        assert stacked.shape[0] == 513
    else:
        stacked = Wp
        assert stacked.shape[0] == 512
    return stacked


def _to_bf16(x):
    import ml_dtypes
    return np.ascontiguousarray(np.asarray(x)).astype(ml_dtypes.bfloat16)


def _get_fast_runner(nc, n_cores=8):
    """Compile the SPMD program through bass2jax's PJRT path once; cache the
    jitted executable so repeated kernel() calls skip retrace/recompile."""
    global _FAST
    if _FAST is not None and _FAST[0] is nc:
        return _FAST[1:]
    import jax
    from concourse import bass2jax as b2j

    b2j.install_neuronx_cc_hook()
    partition_name = (nc.partition_id_tensor.name
                      if nc.partition_id_tensor else None)
    in_names, out_names, out_avals, zero_outs = [], [], [], []
    for alloc in nc.m.functions[0].allocations:
        if not isinstance(alloc, mybir.MemoryLocationSet):
            continue
        name = alloc.memorylocations[0].name
        if alloc.kind == "ExternalInput":
            if name != partition_name:
                in_names.append(name)
        elif alloc.kind == "ExternalOutput":
            assert alloc.tensor_shape is not None and alloc.dtype is not None
            out_names.append(name)
            shape = tuple(alloc.tensor_shape)
            dtype = mybir.dt.np(alloc.dtype)
            out_avals.append(jax.core.ShapedArray(shape, dtype))
            zero_outs.append(np.zeros(shape, dtype))
    n_params = len(in_names)
    all_names = list(in_names) + list(out_names)
    if partition_name is not None:
        all_names.append(partition_name)
    donate = tuple(range(n_params, n_params + len(out_names)))

    def _body(*args):
        operands = list(args)
        if partition_name is not None:
            operands.append(b2j.partition_id_tensor())
        outs = b2j._bass_exec_p.bind(
            *operands,
            out_avals=tuple(out_avals),
            in_names=tuple(all_names),
            out_names=tuple(out_names),
            lowering_input_output_aliases=(),
            sim_require_finite=True,
            sim_require_nnan=True,
            nc=nc,
        )
        return tuple(outs)

    devices = jax.devices()[:n_cores]
    assert len(devices) == n_cores
    mesh = b2j.Mesh(np.asarray(devices), ("core",))
    in_specs = (b2j.PartitionSpec("core"),) * (n_params + len(out_names))
    out_specs = (b2j.PartitionSpec("core"),) * len(out_names)
    sharded = jax.jit(
        b2j.shard_map(_body, mesh=mesh, in_specs=in_specs,
                      out_specs=out_specs, check_rep=False),
        donate_argnums=donate, keep_unused=True)
    _FAST = (nc, sharded, in_names, out_names, out_avals, zero_outs, mesh)
    return _FAST[1:]


def _run_fast(nc, in_maps, n_cores=8):
    """Place inputs on the cores, then time a warm execution of the NEFF.

    Returns (per-core result dicts, warm_exec_wall_s, upload_wall_s)."""
    global LAST_RUN_WALL_S, LAST_UPLOAD_WALL_S
    import jax
    from jax.sharding import NamedSharding, PartitionSpec

    sharded, in_names, out_names, out_avals, zero_outs, mesh = \
        _get_fast_runner(nc, n_cores)
    shard = NamedSharding(mesh, PartitionSpec("core"))

    t0 = _time.perf_counter()
    concat_in = [
        np.concatenate([np.asarray(m[name]) for m in in_maps], axis=0)
        for name in in_names
    ]
    dev_in = [jax.device_put(a, shard) for a in concat_in]
    for a in dev_in:
        a.block_until_ready()
    LAST_UPLOAD_WALL_S = _time.perf_counter() - t0

    def _zeros_dev():
        zs = [jax.device_put(
                  np.zeros((n_cores * z.shape[0], *z.shape[1:]), z.dtype),
                  shard)
              for z in zero_outs]
        for z in zs:
            z.block_until_ready()
        return zs

    # warm-up execution (compiles on the first kernel() call)
    outs = sharded(*dev_in, *_zeros_dev())
    jax.block_until_ready(outs)

    # one blocked execution: upper bound incl. the fixed axon relay RTT
    zeros2 = _zeros_dev()
    t0 = _time.perf_counter()
    outs = sharded(*dev_in, *zeros2)
    jax.block_until_ready(outs)
    t_single = _time.perf_counter() - t0
    LAST_RUN_WALL_S = t_single

    # amortized on-device execution time: N back-to-back executions
    # (async dispatch, executions serialize on the cores), slope vs the
    # single call removes the fixed relay round-trip. This is the
    # host-side estimate of the NEFF execution time a neuron-profile
    # trace would report (NTFF capture is unavailable under axon here).
    global LAST_EXEC_NS
    LAST_EXEC_NS = None
    try:
        NBATCH = 8
        zsets = [_zeros_dev() for _ in range(NBATCH)]
        t0 = _time.perf_counter()
        for zs in zsets:
            outs = sharded(*dev_in, *zs)
        jax.block_until_ready(outs)
        t_batch = _time.perf_counter() - t0
        per_exec = (t_batch - t_single) / (NBATCH - 1)
        if 0.0 < per_exec < t_single:
            LAST_EXEC_NS = int(per_exec * 1e9)
    except Exception:
        pass

    results = []
    host_outs = [np.asarray(o) for o in outs]
    for c in range(n_cores):
        results.append({
            name: host_outs[i].reshape(n_cores, *out_avals[i].shape)[c]
            for i, name in enumerate(out_names)
        })
    return results


def kernel(**inputs):
    import ml_dtypes
    global LAST_RESULTS, LAST_RUN_WALL_S, LAST_UPLOAD_WALL_S, LAST_EXEC_NS
    LAST_RESULTS = None
    LAST_RUN_WALL_S = None
    LAST_UPLOAD_WALL_S = None
    LAST_EXEC_NS = None
    words = np.asarray(inputs["words"])
    capitals = np.asarray(inputs["capitals"])
    B, T = words.shape
    assert B == B_FULL

    perm = _gate_perm()
    cap_table = np.asarray(inputs["cap_table"], np.float32)
    # biases of layers 1/2 are usually all-zero; then the only bias is the
    # +1.0 forget bias, applied for free via the ScalarE activation bias,
    # and the per-step bias matmuls are dropped entirely
    hb = any(np.any(np.asarray(inputs[k], np.float32) != 0.0)
             for k in ("bf1", "bf2", "bb1", "bb2"))
    nc = _get_program(T, hb)

    w_by_dir = [
        [_prep_lstm_w(inputs["Wf0"], inputs["bf0"], cap_table, perm, True, hb),
         _prep_lstm_w(inputs["Wf1"], inputs["bf1"], cap_table, perm, False, hb),
         _prep_lstm_w(inputs["Wf2"], inputs["bf2"], cap_table, perm, False, hb)],
        [_prep_lstm_w(inputs["Wb0"], inputs["bb0"], cap_table, perm, True, hb),
         _prep_lstm_w(inputs["Wb1"], inputs["bb1"], cap_table, perm, False, hb),
         _prep_lstm_w(inputs["Wb2"], inputs["bb2"], cap_table, perm, False, hb)],
    ]
    w_bf = [[_to_bf16(w) for w in ws] for ws in w_by_dir]

    emb_bf = np.asarray(inputs["embed_words"], np.float32).astype(
        ml_dtypes.bfloat16)
    d1w_np = _to_bf16(inputs["d1_W"])
    d1b_np = _to_bf16(np.asarray(inputs["d1_b"])[None, :])
    d2w_np = np.ascontiguousarray(np.asarray(inputs["d2_W"], np.float32))
    d2b_np = np.ascontiguousarray(np.asarray(inputs["d2_b"], np.float32)[None, :])

    in_maps = []
    for p in range(8):
        d, q = p // 4, p % 4
        wl = words[BQ * q:BQ * (q + 1)]
        cl = capitals[BQ * q:BQ * (q + 1)]
        if d == 1:
            wl = wl[:, ::-1]
            cl = cl[:, ::-1]
        # t-major token order r = t*BQ + b; host-side embedding gather,
        # laid out feature-major for the PE's stationary operand
        wflat = np.ascontiguousarray(wl.T).reshape(-1)
        g = emb_bf[wflat]                                   # [TOK, 200] bf16
        xta_np = np.ascontiguousarray(g[:, 0:128].T)        # [128, TOK]
        tokn = wflat.shape[0]
        cflat = np.ascontiguousarray(cl.T).reshape(-1)
        xtb_np = np.empty((77, tokn), ml_dtypes.bfloat16)
        xtb_np[0:72] = g[:, 128:200].T
        xtb_np[72:76] = (cflat[None, :] == np.arange(4)[:, None]).astype(
            ml_dtypes.bfloat16)
        xtb_np[76] = ml_dtypes.bfloat16(1.0)

        in_maps.append({
            "xta": xta_np,
            "xtb": xtb_np,
            "w0": w_bf[d][0],
            "w1": w_bf[d][1],
            "w2": w_bf[d][2],
            "d1w": d1w_np,
            "d1b": d1b_np,
            "d2w": d2w_np,
            "d2b": d2b_np,
        })

    try:
        results = _run_fast(nc, in_maps, n_cores=8)
        return np.ascontiguousarray(results[0]["out"].T.astype(np.float32))
    except Exception:
        import traceback
        traceback.print_exc()
        # fall back to the stock SPMD runner (cold path, correct but slower)
        t0 = _time.time()
        res = run_bass_kernel_spmd(nc, in_maps, core_ids=list(range(8)))
        LAST_RUN_WALL_S = _time.time() - t0
        LAST_RESULTS = res
        return np.ascontiguousarray(res.results[0]["out"].T.astype(np.float32))
